# revision 18
# baseline (speedup 1.0000x reference)
"""Trainium2 Bass kernel for nn_BinaryCNN (5 convs + 3 FCs, W8A16 fake-quant,
BN folded, hardtanh). Pure data parallelism: batch 1024 sharded 128/core over
8 NeuronCores; weights replicated.

Per-core layout strategy:
  - Conv feature maps live in SBUF as [channels(partitions), img, Hp, Wp]
    fully-padded grids in fp16; convolution = sum over kernel positions of
    accumulating matmuls whose moving operand is a *contiguous shifted slice*
    of the grid (garbage is computed at pad positions and never read).
  - conv1 output is written 4x-replicated in partitions (M-replicated
    weights), conv2 output 2x-replicated, so conv2/conv3 run 4/2 concurrent
    row-tiled matmul streams (K=32/K=64); supersteps interleave the streams
    across PSUM banks so fills overlap (measured 62ns/MM vs 420ns serial).
  - Every BN affine is folded algebraically into the *next* layer's weights:
    stored activations are clip(pool(raw), per-channel bounds), grid pads
    hold -T/S so the fold stays exact. Epilogues are then just max-pool
    (DVE reduce+max from PSUM) and one clip+fp16-downcast (DVE tensor_scalar).
  - After conv5, activations sit as [128, half, s, img]; the 3 FC layers run
    with batch=128 moving, streaming pre-transposed folded weights from DRAM,
    8 PSUM banks accumulating in parallel.
"""

import os
import sys

for _p in ("/opt/trn_rl_repo", "/root/.axon_site/_ro/trn_rl_repo"):
    if os.path.isdir(_p) and _p not in sys.path:
        sys.path.append(_p)

import numpy as np

F16 = np.float16
EPS = np.float32(1e-5)
NCORES = 8
BCORE = 128
BT = 16  # images per batch-tile


def _wq(w):
    """numpy replica of reference._wq (per-out-channel symmetric int8 fake quant)."""
    w = np.asarray(w, np.float32)
    s = np.max(np.abs(w), axis=tuple(range(1, w.ndim)), keepdims=True) / np.float32(127.0)
    s = np.maximum(s, np.float32(1e-8)).astype(np.float32)
    q = np.clip(np.round(w / s), -128.0, 127.0).astype(np.float32)
    return (q * s).astype(np.float32)


def _bn_fold(b, g, be, mu, va):
    """(conv_raw + b) -> BN  ==  conv_raw * s + t."""
    inv = (np.float32(1.0) / np.sqrt(np.asarray(va, np.float32) + EPS)).astype(np.float32)
    s = (np.asarray(g, np.float32) * inv).astype(np.float32)
    t = ((np.asarray(b, np.float32) - np.asarray(mu, np.float32)) * s
         + np.asarray(be, np.float32)).astype(np.float32)
    return s, t


def _prep_consts(inp):
    """Fake-quant + fold the whole affine chain into weights/bounds.

    Stored activation st_i = clip(pool(r_i), LO_i, HI_i) where r_i is the PE
    result with folded weights W_i' = w_iq * S_{i-1}[ci]; the true activation
    is h_i = S_i*st_i + T_i with S_i = bn_scale_i and
    T_i = bn_scale_i * c_i + bn_shift_i, c_i = sum_{ci,kh,kw} w_iq * T_{i-1}.
    Grid pads hold -T/S so padded positions represent h=0 exactly.
    Requires S_i > 0 (holds: g_i are ones, rsqrt > 0).
    """
    w1q = _wq(inp["w1"])
    w2q = _wq(inp["w2"])
    w3q = _wq(inp["w3"])
    w4q = _wq(inp["w4"])
    w5q = _wq(inp["w5"])
    fw1q = _wq(inp["fw1"])
    fw2q = _wq(inp["fw2"])
    fw3q = _wq(inp["fw3"])

    sb, tb = {}, {}
    for i in range(1, 6):
        sb[i], tb[i] = _bn_fold(inp[f"b{i}"], inp[f"g{i}"], inp[f"be{i}"],
                                inp[f"mu{i}"], inp[f"va{i}"])
    for j in range(1, 3):
        sb[5 + j], tb[5 + j] = _bn_fold(inp[f"fb{j}"], inp[f"fg{j}"], inp[f"fbe{j}"],
                                        inp[f"fmu{j}"], inp[f"fva{j}"])
    assert all(np.all(sb[i] > 0) for i in sb), "BN scale must be positive for pool/clip fold"

    S, T, LO, HI = {}, {}, {}, {}

    def layer(i, wq, s_prev, t_prev):
        # wq [co, ci, kh, kw]; returns folded weights W' and sets S/T/LO/HI[i]
        Wp = (wq * s_prev[None, :, None, None]).astype(np.float32)
        c = np.einsum("ocij,c->o", wq, t_prev).astype(np.float32)
        S[i] = sb[i]
        T[i] = (sb[i] * c + tb[i]).astype(np.float32)
        LO[i] = ((-1.0 - T[i]) / S[i]).astype(np.float32)
        HI[i] = ((1.0 - T[i]) / S[i]).astype(np.float32)
        return Wp

    W1p = layer(1, w1q, np.ones(1, np.float32), np.zeros(1, np.float32))
    W2p = layer(2, w2q, S[1], T[1])
    W3p = layer(3, w3q, S[2], T[2])
    W4p = layer(4, w4q, S[3], T[3])
    W5p = layer(5, w5q, S[4], T[4])

    def fc_layer(i, wq, s_prev_vec, t_prev_vec, bn=True):
        Wp = (wq * s_prev_vec[None, :]).astype(np.float32)
        c = (wq @ t_prev_vec).astype(np.float32)
        if bn:
            S[i] = sb[i]
            T[i] = (sb[i] * c + tb[i]).astype(np.float32)
            LO[i] = ((-1.0 - T[i]) / S[i]).astype(np.float32)
            HI[i] = ((1.0 - T[i]) / S[i]).astype(np.float32)
        else:
            S[i], T[i] = None, c  # fc3: out = r + c + fb3
        return Wp

    s5v = np.repeat(S[5], 9).astype(np.float32)  # fi = ch*9 + s
    t5v = np.repeat(T[5], 9).astype(np.float32)
    FW1p = fc_layer(6, fw1q, s5v, t5v)
    FW2p = fc_layer(7, fw2q, np.repeat(S[6], 1), T[6])
    FW3p = fc_layer(8, fw3q, S[7], T[7], bn=False)
    bias3 = (np.asarray(inp["fb3"], np.float32) + T[8]).astype(np.float32)

    # ---- device layouts ----
    w1m = W1p.reshape(32, 25).T  # [kidx, co]
    W1R = np.zeros((128, 128), np.float32)
    blk = np.tile(w1m, (1, 4))  # M-replicate x4 -> conv1 out [4x32co]
    for g in range(4):
        W1R[32 * g:32 * g + 25, :] = blk

    w2m = W2p.reshape(64, 32, 25).transpose(1, 2, 0)  # [ci, pos, co]
    w2m = np.tile(w2m, (1, 1, 2))  # M-replicate x2 -> conv2 out [2x64co]
    W2R = np.zeros((128, 25, 128), np.float32)
    for g in range(4):
        W2R[32 * g:32 * g + 32] = w2m

    w3m = W3p.reshape(128, 64, 9).transpose(1, 2, 0)  # [64, 9, 128]
    W3R = np.zeros((128, 9, 128), np.float32)
    for g in range(2):
        W3R[64 * g:64 * g + 64] = w3m

    W4D = W4p.reshape(256, 128, 9).transpose(1, 2, 0).copy()  # [ci, pos, co]
    W5D = W5p.reshape(256, 256).T.reshape(2, 128, 256).transpose(1, 0, 2).copy()

    FW1D = (FW1p.reshape(8, 128, 2, 128, 9)  # [mt, j, cc, cl, s]
            .transpose(4, 2, 3, 0, 1).reshape(18, 128, 1024).copy())
    FW2D = (FW2p.reshape(4, 128, 8, 128)  # [mt, j, kt, c]
            .transpose(2, 3, 0, 1).reshape(8, 128, 512).copy())
    FW3D = np.ascontiguousarray(FW3p.reshape(10, 4, 128).transpose(1, 2, 0))

    p = np.arange(128)
    AFF = np.zeros((128, 44), np.float32)
    AFF[:, 0] = LO[1][p % 32]
    AFF[:, 1] = HI[1][p % 32]
    AFF[:, 2] = LO[2][p % 64]
    AFF[:, 3] = HI[2][p % 64]
    AFF[:, 4] = LO[3]
    AFF[:, 5] = HI[3]
    for h in range(2):
        AFF[:, 6 + h] = LO[4][128 * h + p]
        AFF[:, 8 + h] = HI[4][128 * h + p]
        AFF[:, 10 + h] = LO[5][128 * h + p]
        AFF[:, 12 + h] = HI[5][128 * h + p]
    for mt in range(8):
        AFF[:, 14 + mt] = LO[6][128 * mt + p]
        AFF[:, 22 + mt] = HI[6][128 * mt + p]
    for mt in range(4):
        AFF[:, 30 + mt] = LO[7][128 * mt + p]
        AFF[:, 34 + mt] = HI[7][128 * mt + p]
    AFF[:10, 38] = bias3
    AFF[:, 39] = (-T[1] / S[1])[p % 32]   # h1 grid pad value per channel
    AFF[:, 40] = (-T[2] / S[2])[p % 64]   # h3 grid pad
    AFF[:, 41] = (-T[3] / S[3])[p]        # h4 grid pad

    return {
        "W1R": W1R.astype(F16), "W2R": W2R.astype(F16), "W3R": W3R.astype(F16),
        "W4P": W4D.astype(F16), "W5P": W5D.astype(F16),
        "FW1": FW1D.astype(F16), "FW2": FW2D.astype(F16), "FW3": FW3D.astype(F16),
        "AFF": AFF,
    }


def _prep_xi(x_core):
    """Host im2col for conv1 (CI=1): [25, nimg*1024] fp16 on the 32x32 grid."""
    nimg = x_core.shape[0]
    xp = np.zeros((nimg, 36, 36), np.float32)
    xp[:, 2:30, 2:30] = np.asarray(x_core, np.float32).reshape(nimg, 28, 28)
    xi = np.stack([xp[:, kh:kh + 32, kw:kw + 32]
                   for kh in range(5) for kw in range(5)], axis=0)
    return np.ascontiguousarray(xi.reshape(25, nimg * 1024)).astype(F16)


_NC_CACHE = {}


def _build_module(nbt):
    from concourse import bacc
    import concourse.mybir as mybir
    from concourse.tile import TileContext

    f32 = mybir.dt.float32
    f16 = mybir.dt.float16
    AX = mybir.AxisListType.X
    XY = mybir.AxisListType.XY
    MAX = mybir.AluOpType.max
    MIN = mybir.AluOpType.min
    IDENT = mybir.ActivationFunctionType.Identity

    bc = nbt * BT
    nc = bacc.Bacc(None, target_bir_lowering=False, debug=False)

    XI = nc.dram_tensor("XI", [25, bc * 1024], f16, kind="ExternalInput")
    W1 = nc.dram_tensor("W1R", [128, 128], f16, kind="ExternalInput")
    W2 = nc.dram_tensor("W2R", [128, 25, 128], f16, kind="ExternalInput")
    W3 = nc.dram_tensor("W3R", [128, 9, 128], f16, kind="ExternalInput")
    W4 = nc.dram_tensor("W4P", [128, 9, 256], f16, kind="ExternalInput")
    W5 = nc.dram_tensor("W5P", [128, 2, 256], f16, kind="ExternalInput")
    FW1 = nc.dram_tensor("FW1", [18, 128, 1024], f16, kind="ExternalInput")
    FW2 = nc.dram_tensor("FW2", [8, 128, 512], f16, kind="ExternalInput")
    FW3 = nc.dram_tensor("FW3", [4, 128, 10], f16, kind="ExternalInput")
    AFF = nc.dram_tensor("AFF", [128, 44], f32, kind="ExternalInput")
    OUT = nc.dram_tensor("OUT", [bc, 10], f32, kind="ExternalOutput")

    with TileContext(nc) as tc:
        with (
            tc.tile_pool(name="wp", bufs=1) as wp,
            tc.tile_pool(name="bigp", bufs=1) as bigp,
            tc.tile_pool(name="actp", bufs=2) as actp,
            tc.tile_pool(name="tmpp", bufs=4) as tmpp,
            tc.tile_pool(name="fwp", bufs=8) as fwp,
            tc.tile_pool(name="psp", bufs=8, space="PSUM") as psp,
        ):
            # ---- resident weights / constants ----
            # (w1 + aff + the first im2col slab go first so conv1 of the first
            #  batch-tile can start while the bulk weights stream in)
            w1_sb = wp.tile([128, 128], f16)
            nc.sync.dma_start(w1_sb[:], W1[:])
            xi0 = bigp.tile([128, BT * 1024], f16, name="xi_sb")
            for g in range(4):
                nc.sync.dma_start(xi0[32 * g:32 * g + 25, 0:8192], XI[:, 0:8192])
            w2_sb = wp.tile([128, 25, 128], f16)
            nc.sync.dma_start(w2_sb[:], W2[:])
            w3_sb = wp.tile([128, 9, 128], f16)
            nc.sync.dma_start(w3_sb[:], W3[:])
            w4_sb = wp.tile([128, 9, 256], f16)
            nc.sync.dma_start(w4_sb[:], W4[:])
            w5_sb = wp.tile([128, 2, 256], f16)
            nc.sync.dma_start(w5_sb[:], W5[:])
            aff = wp.tile([128, 44], f32)
            nc.sync.dma_start(aff[:], AFF[:])
            fw3_sb = wp.tile([128, 4, 10], f16)
            for kt in range(4):
                nc.sync.dma_start(fw3_sb[:, kt, :], FW3[kt])
            for g in range(4):
                nc.sync.dma_start(xi0[32 * g:32 * g + 25, 8192:16384],
                                  XI[:, 8192:16384])

            h5f = bigp.tile([128, 2, 9, bc], f16)  # conv5 out == FC1 rhs

            for bti in range(nbt):
                if bti == 0:
                    xi_sb = xi0
                else:
                    xi_sb = bigp.tile([128, BT * 1024], f16, name="xi_sb")
                    for q in range(BT // 8):
                        for g in range(4):
                            nc.sync.dma_start(
                                xi_sb[32 * g:32 * g + 25, q * 8192:(q + 1) * 8192],
                                XI[:, bti * BT * 1024 + q * 8192:
                                   bti * BT * 1024 + (q + 1) * 8192])

                # ---- grids; pads carry the per-channel -T/S fold value.
                # Pads are identical each iteration, so fill them only on the
                # first pass through each of the 2 rotating slots.
                h1 = actp.tile([128, BT * 1024], f16, name="h1")
                h1v = h1.rearrange("p (i r w) -> p i r w", r=32, w=32)
                h3 = actp.tile([128, BT * 256], f16, name="h3")
                h3v = h3.rearrange("p (i r w) -> p i r w", r=16, w=16)
                h4 = actp.tile([128, BT * 81], f16, name="h4")
                h4v = h4.rearrange("p (i r w) -> p i r w", r=9, w=9)
                h5t = actp.tile([128, 2, BT, 3, 3], f16, name="h5t")
                if bti < 2:
                    pc = aff[:, 39:40]
                    nc.gpsimd.tensor_copy(h1v[:, :, 0:2, :], pc.to_broadcast((128, BT, 2, 32)))
                    nc.gpsimd.tensor_copy(h1v[:, :, 30:32, :], pc.to_broadcast((128, BT, 2, 32)))
                    nc.gpsimd.tensor_copy(h1v[:, :, 2:30, 0:2], pc.to_broadcast((128, BT, 28, 2)))
                    nc.gpsimd.tensor_copy(h1v[:, :, 2:30, 30:32], pc.to_broadcast((128, BT, 28, 2)))
                    pc = aff[:, 40:41]
                    nc.gpsimd.tensor_copy(h3v[:, :, 0:1, :], pc.to_broadcast((128, BT, 1, 16)))
                    nc.gpsimd.tensor_copy(h3v[:, :, 15:16, :], pc.to_broadcast((128, BT, 1, 16)))
                    nc.gpsimd.tensor_copy(h3v[:, :, 1:15, 0:1], pc.to_broadcast((128, BT, 14, 1)))
                    nc.gpsimd.tensor_copy(h3v[:, :, 1:15, 15:16], pc.to_broadcast((128, BT, 14, 1)))
                    pc = aff[:, 41:42]
                    nc.gpsimd.tensor_copy(h4v[:, :, 0:1, :], pc.to_broadcast((128, BT, 1, 9)))
                    nc.gpsimd.tensor_copy(h4v[:, :, 8:9, :], pc.to_broadcast((128, BT, 1, 9)))
                    nc.gpsimd.tensor_copy(h4v[:, :, 1:8, 0:1], pc.to_broadcast((128, BT, 7, 1)))
                    nc.gpsimd.tensor_copy(h4v[:, :, 1:8, 8:9], pc.to_broadcast((128, BT, 7, 1)))

                # ==== conv1 (whole tile), then conv2 ====
                xiv = xi_sb.rearrange("p (i r w) -> p i r w", r=32, w=32)
                for ss in range(BT // 4):
                    for cb in range(2):
                        ps1 = [psp.tile([128, 392], f32, name=f"ps1_{cb}_{g}",
                                        tag="ps") for g in range(4)]
                        for g in range(4):
                            img = ss * 4 + g
                            nc.tensor.matmul(
                                ps1[g][:],
                                w1_sb[32 * g:32 * g + 25, :],
                                xiv[32 * g:32 * g + 25, img, 14 * cb:14 * cb + 14, 0:28],
                                start=True, stop=True, tile_position=(32 * g, 0))
                        for g in range(4):
                            img = ss * 4 + g
                            dst = h1v[:, img, 2 + 14 * cb:16 + 14 * cb, 2:30]
                            psr = ps1[g].rearrange("p (r w) -> p r w", w=28)
                            nc.scalar.copy(dst, psr)
                            nc.vector.tensor_scalar(dst, dst, aff[:, 0:1],
                                                    aff[:, 1:2], MAX, MIN)
                for ss in range(BT // 4):
                    hps = {}
                    for cb in range(2):
                        ps2 = [psp.tile([128, 392], f32, name=f"ps2_{cb}_{g}",
                                        tag="ps") for g in range(4)]
                        for pos in range(25):
                            kh, kw = divmod(pos, 5)
                            for g in range(4):
                                img = ss * 4 + g
                                nc.tensor.matmul(
                                    ps2[g][:],
                                    w2_sb[32 * g:32 * g + 32, pos, :],
                                    h1v[32 * g:32 * g + 32, img,
                                        14 * cb + kh:14 * cb + kh + 14, kw:kw + 28],
                                    start=(pos == 0), stop=(pos == 24),
                                    tile_position=(32 * g, 0))
                        for g in range(4):
                            img = ss * 4 + g
                            if cb == 0:
                                hps[g] = tmpp.tile([128, 14, 14], f16, name="hp2",
                                                   bufs=6)
                            psv = ps2[g].rearrange(
                                "p (y ty x tx) -> p y x ty tx", y=7, ty=2, tx=2)
                            nc.vector.tensor_reduce(
                                hps[g][:, 7 * cb:7 * cb + 7, :], psv[:],
                                axis=XY, op=MAX)
                            if cb == 1:
                                nc.vector.tensor_scalar(
                                    h3v[:, img, 1:15, 1:15], hps[g][:],
                                    aff[:, 2:3], aff[:, 3:4], MAX, MIN)

                # ======== conv3: 9 shifted MMs, 2 strips x 2 banks/strip ========
                for ss in range(2):
                    pss = [psp.tile([128, 392], f32, name=f"ps3_{g}_{cb}", tag="ps")
                           for g in range(2) for cb in range(2)]
                    for pos in range(9):
                        kh, kw = divmod(pos, 3)
                        for g in range(2):
                            for cb in range(2):
                                i0 = ss * 8 + 4 * cb + 2 * g  # first of 2 images
                                nc.tensor.matmul(
                                    pss[2 * g + cb][:],
                                    w3_sb[64 * g:64 * g + 64, pos, :],
                                    h3v[64 * g:64 * g + 64, i0:i0 + 2,
                                        kh:kh + 14, kw:kw + 14],
                                    start=(pos == 0), stop=(pos == 8),
                                    tile_position=(64 * g, 0))
                    for g in range(2):
                        for cb in range(2):
                            i0 = ss * 8 + 4 * cb + 2 * g
                            psv = pss[2 * g + cb].rearrange(
                                "p (m ty x tx) -> p m x ty tx", m=14, ty=2, tx=2)
                            hp3 = tmpp.tile([128, 14, 7], f16, name="hp3")
                            nc.vector.tensor_reduce(hp3[:], psv[:], axis=XY, op=MAX)
                            nc.vector.tensor_scalar(
                                h4v[:, i0:i0 + 2, 1:8, 1:8],
                                hp3.rearrange("p (i y) x -> p i y x", i=2),
                                aff[:, 4:5], aff[:, 5:6], MAX, MIN)

                # ======== conv4: K=128, M=256 in halves, 3 psum chunks ========
                pse = {(mh, img0): psp.tile([128, 490], f32,
                                            name=f"ps4_{mh}_{img0}", tag="ps")
                       for mh in range(2) for img0 in (0, 10)}
                for pos in range(9):
                    kh, kw = divmod(pos, 3)
                    for mh in range(2):
                        for (img0, nimg) in ((0, 10), (10, 6)):
                            nc.tensor.matmul(
                                pse[(mh, img0)][:, 0:nimg * 49],
                                w4_sb[:, pos, 128 * mh:128 * mh + 128],
                                h4v[:, img0:img0 + nimg, kh:kh + 7, kw:kw + 7],
                                start=(pos == 0), stop=(pos == 8))
                for mh in range(2):
                    for (img0, nimg) in ((0, 10), (10, 6)):
                        psv = pse[(mh, img0)][:, 0:nimg * 49].rearrange(
                            "p (i r w) -> p i r w", r=7, w=7)
                        tw4 = tmpp.tile([128, 10, 6, 3], f16, name="tw4")
                        nc.vector.tensor_reduce(
                            tw4[:, 0:nimg],
                            psv[:, :, 0:6, 0:6].rearrange("p i r (x t) -> p i r x t", t=2),
                            axis=AX, op=MAX)
                        twp4 = tw4[:, 0:nimg].rearrange("p i (r t) x -> p i r t x", t=2)
                        hp4 = tmpp.tile([128, 10, 3, 3], f16, name="hp4")
                        nc.vector.tensor_tensor(hp4[:, 0:nimg], twp4[:, :, :, 0, :],
                                                twp4[:, :, :, 1, :], MAX)
                        nc.vector.tensor_scalar(
                            h5t[:, mh, img0:img0 + nimg, :, :], hp4[:, 0:nimg],
                            aff[:, 6 + mh:7 + mh], aff[:, 8 + mh:9 + mh], MAX, MIN)

                # ======== conv5 (1x1): K=256 in 2 chunks, M=256 in halves ======
                for mh in range(2):
                    ps5 = psp.tile([128, BT * 9], f32, name="ps5", tag="ps")
                    for kc in range(2):
                        nc.tensor.matmul(
                            ps5[:],
                            w5_sb[:, kc, 128 * mh:128 * mh + 128],
                            h5t[:, kc, :, :, :],
                            start=(kc == 0), stop=(kc == 1))
                    nc.vector.tensor_scalar(
                        h5f[:, mh, :, bti * BT:(bti + 1) * BT],
                        ps5.rearrange("p (i s) -> p s i", s=9),
                        aff[:, 10 + mh:11 + mh], aff[:, 12 + mh:13 + mh], MAX, MIN)

            # ================= FC phase (batch = bc) =================
            z1 = bigp.tile([128, 8, bc], f16)
            fpt = [psp.tile([128, bc], f32, name=f"fpt{i}", tag="ps")
                   for i in range(8)]
            fps = [fpt[mt][:] for mt in range(8)]
            for kt in range(18):
                s_, cc = divmod(kt, 2)
                fw1t = fwp.tile([128, 1024], f16, name="fw1t")
                nc.gpsimd.dma_start(fw1t[:], FW1[kt])
                for mt in range(8):
                    nc.tensor.matmul(fps[mt], fw1t[:, 128 * mt:128 * mt + 128],
                                     h5f[:, cc, s_, :],
                                     start=(kt == 0), stop=(kt == 17))
            for mt in range(8):
                nc.vector.tensor_scalar(z1[:, mt, :], fps[mt],
                                        aff[:, 14 + mt:15 + mt],
                                        aff[:, 22 + mt:23 + mt], MAX, MIN)

            z2 = bigp.tile([128, 4, bc], f16)
            f2t = [psp.tile([128, bc], f32, name=f"f2t{i}", tag="ps") for i in range(4)]
            f2s = [f2t[mt][:] for mt in range(4)]
            for kt in range(8):
                fw2t = fwp.tile([128, 512], f16, name="fw2t")
                nc.gpsimd.dma_start(fw2t[:], FW2[kt])
                for mt in range(4):
                    nc.tensor.matmul(f2s[mt], fw2t[:, 128 * mt:128 * mt + 128],
                                     z1[:, kt, :],
                                     start=(kt == 0), stop=(kt == 7))
            for mt in range(4):
                nc.vector.tensor_scalar(z2[:, mt, :], f2s[mt],
                                        aff[:, 30 + mt:31 + mt],
                                        aff[:, 34 + mt:35 + mt], MAX, MIN)

            f3ps = psp.tile([10, bc], f32, name="f3ps", tag="ps")
            for kt in range(4):
                nc.tensor.matmul(f3ps[:], fw3_sb[:, kt, :], z2[:, kt, :],
                                 start=(kt == 0), stop=(kt == 3))
            out_sb = tmpp.tile([10, bc], f32, name="out_sb")
            nc.scalar.activation(out_sb[:], f3ps[:], IDENT,
                                 bias=aff[0:10, 38:39], scale=1.0)
            nc.sync.dma_start(OUT[:].rearrange("b c -> c b"), out_sb[:])

    nc.compile()
    return nc


def _get_nc(nbt):
    if nbt not in _NC_CACHE:
        _NC_CACHE[nbt] = _build_module(nbt)
    return _NC_CACHE[nbt]


def _make_in_maps(inputs, ncores=NCORES, bc=BCORE):
    consts = _prep_consts(inputs)
    x = np.asarray(inputs["x"], np.float32)
    in_maps = []
    for c in range(ncores):
        m = dict(consts)
        m["XI"] = _prep_xi(x[c * bc:(c + 1) * bc])
        in_maps.append(m)
    return in_maps


def run(inputs, trace=False, tmpdir=None, ncores=NCORES):
    from concourse.bass_utils import run_bass_kernel_spmd

    nbt = BCORE // BT
    nc = _get_nc(nbt)
    in_maps = _make_in_maps(inputs, ncores=ncores)
    kw = {}
    if trace:
        if tmpdir is not None:
            os.makedirs(tmpdir, exist_ok=True)
        kw = dict(trace=True, tmpdir=tmpdir)
    res = run_bass_kernel_spmd(nc, in_maps, core_ids=list(range(ncores)), **kw)
    out = np.concatenate([res.results[c]["OUT"] for c in range(ncores)], axis=0)
    return out.astype(np.float32), res


def kernel(**inputs):
    out, _ = run(inputs)
    return out


# revision 19
# speedup vs baseline: 1.0379x; 1.0379x over previous
"""Trainium2 Bass kernel for nn_BinaryCNN (5 convs + 3 FCs, W8A16 fake-quant,
BN folded, hardtanh). Pure data parallelism: batch 1024 sharded 128/core over
8 NeuronCores; weights replicated.

Per-core layout strategy:
  - Conv feature maps live in SBUF as [channels(partitions), img, Hp, Wp]
    fully-padded grids in fp16; convolution = sum over kernel positions of
    accumulating matmuls whose moving operand is a *contiguous shifted slice*
    of the grid (garbage is computed at pad positions and never read).
  - conv1 output is written 4x-replicated in partitions (M-replicated
    weights), conv2 output 2x-replicated, so conv2/conv3 run 4/2 concurrent
    row-tiled matmul streams (K=32/K=64); supersteps interleave the streams
    across PSUM banks so fills overlap (measured 62ns/MM vs 420ns serial).
  - Every BN affine is folded algebraically into the *next* layer's weights:
    stored activations are clip(pool(raw), per-channel bounds), grid pads
    hold -T/S so the fold stays exact. Epilogues are then just max-pool
    (DVE reduce+max from PSUM) and one clip+fp16-downcast (DVE tensor_scalar).
  - After conv5, activations sit as [128, half, s, img]; the 3 FC layers run
    with batch=128 moving, streaming pre-transposed folded weights from DRAM,
    8 PSUM banks accumulating in parallel.
"""

import os
import sys

for _p in ("/opt/trn_rl_repo", "/root/.axon_site/_ro/trn_rl_repo"):
    if os.path.isdir(_p) and _p not in sys.path:
        sys.path.append(_p)

import numpy as np

F16 = np.float16
EPS = np.float32(1e-5)
NCORES = 8
BCORE = 128
BT = 16  # images per batch-tile


def _wq(w):
    """numpy replica of reference._wq (per-out-channel symmetric int8 fake quant)."""
    w = np.asarray(w, np.float32)
    s = np.max(np.abs(w), axis=tuple(range(1, w.ndim)), keepdims=True) / np.float32(127.0)
    s = np.maximum(s, np.float32(1e-8)).astype(np.float32)
    q = np.clip(np.round(w / s), -128.0, 127.0).astype(np.float32)
    return (q * s).astype(np.float32)


def _bn_fold(b, g, be, mu, va):
    """(conv_raw + b) -> BN  ==  conv_raw * s + t."""
    inv = (np.float32(1.0) / np.sqrt(np.asarray(va, np.float32) + EPS)).astype(np.float32)
    s = (np.asarray(g, np.float32) * inv).astype(np.float32)
    t = ((np.asarray(b, np.float32) - np.asarray(mu, np.float32)) * s
         + np.asarray(be, np.float32)).astype(np.float32)
    return s, t


def _prep_consts(inp):
    """Fake-quant + fold the whole affine chain into weights/bounds.

    Stored activation st_i = clip(pool(r_i), LO_i, HI_i) where r_i is the PE
    result with folded weights W_i' = w_iq * S_{i-1}[ci]; the true activation
    is h_i = S_i*st_i + T_i with S_i = bn_scale_i and
    T_i = bn_scale_i * c_i + bn_shift_i, c_i = sum_{ci,kh,kw} w_iq * T_{i-1}.
    Grid pads hold -T/S so padded positions represent h=0 exactly.
    Requires S_i > 0 (holds: g_i are ones, rsqrt > 0).
    """
    w1q = _wq(inp["w1"])
    w2q = _wq(inp["w2"])
    w3q = _wq(inp["w3"])
    w4q = _wq(inp["w4"])
    w5q = _wq(inp["w5"])
    fw1q = _wq(inp["fw1"])
    fw2q = _wq(inp["fw2"])
    fw3q = _wq(inp["fw3"])

    sb, tb = {}, {}
    for i in range(1, 6):
        sb[i], tb[i] = _bn_fold(inp[f"b{i}"], inp[f"g{i}"], inp[f"be{i}"],
                                inp[f"mu{i}"], inp[f"va{i}"])
    for j in range(1, 3):
        sb[5 + j], tb[5 + j] = _bn_fold(inp[f"fb{j}"], inp[f"fg{j}"], inp[f"fbe{j}"],
                                        inp[f"fmu{j}"], inp[f"fva{j}"])
    assert all(np.all(sb[i] > 0) for i in sb), "BN scale must be positive for pool/clip fold"

    S, T, LO, HI = {}, {}, {}, {}

    def layer(i, wq, s_prev, t_prev):
        # wq [co, ci, kh, kw]; returns folded weights W' and sets S/T/LO/HI[i]
        Wp = (wq * s_prev[None, :, None, None]).astype(np.float32)
        c = np.einsum("ocij,c->o", wq, t_prev).astype(np.float32)
        S[i] = sb[i]
        T[i] = (sb[i] * c + tb[i]).astype(np.float32)
        LO[i] = ((-1.0 - T[i]) / S[i]).astype(np.float32)
        HI[i] = ((1.0 - T[i]) / S[i]).astype(np.float32)
        return Wp

    W1p = layer(1, w1q, np.ones(1, np.float32), np.zeros(1, np.float32))
    W2p = layer(2, w2q, S[1], T[1])
    W3p = layer(3, w3q, S[2], T[2])
    W4p = layer(4, w4q, S[3], T[3])
    W5p = layer(5, w5q, S[4], T[4])

    def fc_layer(i, wq, s_prev_vec, t_prev_vec, bn=True):
        Wp = (wq * s_prev_vec[None, :]).astype(np.float32)
        c = (wq @ t_prev_vec).astype(np.float32)
        if bn:
            S[i] = sb[i]
            T[i] = (sb[i] * c + tb[i]).astype(np.float32)
            LO[i] = ((-1.0 - T[i]) / S[i]).astype(np.float32)
            HI[i] = ((1.0 - T[i]) / S[i]).astype(np.float32)
        else:
            S[i], T[i] = None, c  # fc3: out = r + c + fb3
        return Wp

    s5v = np.repeat(S[5], 9).astype(np.float32)  # fi = ch*9 + s
    t5v = np.repeat(T[5], 9).astype(np.float32)
    FW1p = fc_layer(6, fw1q, s5v, t5v)
    FW2p = fc_layer(7, fw2q, np.repeat(S[6], 1), T[6])
    FW3p = fc_layer(8, fw3q, S[7], T[7], bn=False)
    bias3 = (np.asarray(inp["fb3"], np.float32) + T[8]).astype(np.float32)

    # ---- device layouts ----
    w1m = W1p.reshape(32, 25).T  # [kidx, co]
    W1R = np.zeros((128, 128), np.float32)
    blk = np.tile(w1m, (1, 4))  # M-replicate x4 -> conv1 out [4x32co]
    for g in range(4):
        W1R[32 * g:32 * g + 25, :] = blk

    w2m = W2p.reshape(64, 32, 25).transpose(1, 2, 0)  # [ci, pos, co]
    w2m = np.tile(w2m, (1, 1, 2))  # M-replicate x2 -> conv2 out [2x64co]
    W2R = np.zeros((128, 25, 128), np.float32)
    for g in range(4):
        W2R[32 * g:32 * g + 32] = w2m

    w3m = W3p.reshape(128, 64, 9).transpose(1, 2, 0)  # [64, 9, 128]
    W3R = np.zeros((128, 9, 128), np.float32)
    for g in range(2):
        W3R[64 * g:64 * g + 64] = w3m

    W4D = W4p.reshape(256, 128, 9).transpose(1, 2, 0).copy()  # [ci, pos, co]
    W5D = W5p.reshape(256, 256).T.reshape(2, 128, 256).transpose(1, 0, 2).copy()

    FW1D = (FW1p.reshape(8, 128, 2, 128, 9)  # [mt, j, cc, cl, s]
            .transpose(4, 2, 3, 0, 1).reshape(18, 128, 1024).copy())
    FW2D = (FW2p.reshape(4, 128, 8, 128)  # [mt, j, kt, c]
            .transpose(2, 3, 0, 1).reshape(8, 128, 512).copy())
    FW3D = np.ascontiguousarray(FW3p.reshape(10, 4, 128).transpose(1, 2, 0))

    p = np.arange(128)
    AFF = np.zeros((128, 44), np.float32)
    AFF[:, 0] = LO[1][p % 32]
    AFF[:, 1] = HI[1][p % 32]
    AFF[:, 2] = LO[2][p % 64]
    AFF[:, 3] = HI[2][p % 64]
    AFF[:, 4] = LO[3]
    AFF[:, 5] = HI[3]
    for h in range(2):
        AFF[:, 6 + h] = LO[4][128 * h + p]
        AFF[:, 8 + h] = HI[4][128 * h + p]
        AFF[:, 10 + h] = LO[5][128 * h + p]
        AFF[:, 12 + h] = HI[5][128 * h + p]
    for mt in range(8):
        AFF[:, 14 + mt] = LO[6][128 * mt + p]
        AFF[:, 22 + mt] = HI[6][128 * mt + p]
    for mt in range(4):
        AFF[:, 30 + mt] = LO[7][128 * mt + p]
        AFF[:, 34 + mt] = HI[7][128 * mt + p]
    AFF[:10, 38] = bias3
    AFF[:, 39] = (-T[1] / S[1])[p % 32]   # h1 grid pad value per channel
    AFF[:, 40] = (-T[2] / S[2])[p % 64]   # h3 grid pad
    AFF[:, 41] = (-T[3] / S[3])[p]        # h4 grid pad

    return {
        "W1R": W1R.astype(F16), "W2R": W2R.astype(F16), "W3R": W3R.astype(F16),
        "W4P": W4D.astype(F16), "W5P": W5D.astype(F16),
        "FW1": FW1D.astype(F16), "FW2": FW2D.astype(F16), "FW3": FW3D.astype(F16),
        "AFF": AFF,
    }


def _prep_xi(x_core):
    """Host im2col for conv1 (CI=1): [25, nimg*1024] fp16 on the 32x32 grid."""
    nimg = x_core.shape[0]
    xp = np.zeros((nimg, 36, 36), np.float32)
    xp[:, 2:30, 2:30] = np.asarray(x_core, np.float32).reshape(nimg, 28, 28)
    xi = np.stack([xp[:, kh:kh + 32, kw:kw + 32]
                   for kh in range(5) for kw in range(5)], axis=0)
    return np.ascontiguousarray(xi.reshape(25, nimg * 1024)).astype(F16)


_NC_CACHE = {}


def _build_module(nbt):
    from concourse import bacc
    import concourse.mybir as mybir
    from concourse.tile import TileContext

    f32 = mybir.dt.float32
    f16 = mybir.dt.float16
    AX = mybir.AxisListType.X
    XY = mybir.AxisListType.XY
    MAX = mybir.AluOpType.max
    MIN = mybir.AluOpType.min
    IDENT = mybir.ActivationFunctionType.Identity

    bc = nbt * BT
    nc = bacc.Bacc(None, target_bir_lowering=False, debug=False)

    XI = nc.dram_tensor("XI", [25, bc * 1024], f16, kind="ExternalInput")
    W1 = nc.dram_tensor("W1R", [128, 128], f16, kind="ExternalInput")
    W2 = nc.dram_tensor("W2R", [128, 25, 128], f16, kind="ExternalInput")
    W3 = nc.dram_tensor("W3R", [128, 9, 128], f16, kind="ExternalInput")
    W4 = nc.dram_tensor("W4P", [128, 9, 256], f16, kind="ExternalInput")
    W5 = nc.dram_tensor("W5P", [128, 2, 256], f16, kind="ExternalInput")
    FW1 = nc.dram_tensor("FW1", [18, 128, 1024], f16, kind="ExternalInput")
    FW2 = nc.dram_tensor("FW2", [8, 128, 512], f16, kind="ExternalInput")
    FW3 = nc.dram_tensor("FW3", [4, 128, 10], f16, kind="ExternalInput")
    AFF = nc.dram_tensor("AFF", [128, 44], f32, kind="ExternalInput")
    OUT = nc.dram_tensor("OUT", [bc, 10], f32, kind="ExternalOutput")

    with TileContext(nc) as tc:
        with (
            tc.tile_pool(name="wp", bufs=1) as wp,
            tc.tile_pool(name="bigp", bufs=1) as bigp,
            tc.tile_pool(name="actp", bufs=2) as actp,
            tc.tile_pool(name="tmpp", bufs=4) as tmpp,
            tc.tile_pool(name="fwp", bufs=6) as fwp,
            tc.tile_pool(name="psp", bufs=8, space="PSUM") as psp,
        ):
            # ---- resident weights / constants ----
            # (w1 + aff + the first im2col slab go first so conv1 of the first
            #  batch-tile can start while the bulk weights stream in)
            w1_sb = wp.tile([128, 128], f16)
            nc.sync.dma_start(w1_sb[:], W1[:])
            w2_sb = wp.tile([128, 25, 128], f16)
            nc.sync.dma_start(w2_sb[:], W2[:])
            w3_sb = wp.tile([128, 9, 128], f16)
            nc.sync.dma_start(w3_sb[:], W3[:])
            w4_sb = wp.tile([128, 9, 256], f16)
            nc.sync.dma_start(w4_sb[:], W4[:])
            w5_sb = wp.tile([128, 2, 256], f16)
            nc.sync.dma_start(w5_sb[:], W5[:])
            aff = wp.tile([128, 44], f32)
            nc.sync.dma_start(aff[:], AFF[:])
            fw3_sb = wp.tile([128, 4, 10], f16)
            for kt in range(4):
                nc.sync.dma_start(fw3_sb[:, kt, :], FW3[kt])

            h5f = bigp.tile([128, 2, 9, bc], f16)  # conv5 out == FC1 rhs

            for bti in range(nbt):
                xi_sb = bigp.tile([128, BT * 1024], f16, name="xi_sb")
                for q in range(BT // 8):
                    for g in range(4):
                        nc.sync.dma_start(
                            xi_sb[32 * g:32 * g + 25, q * 8192:(q + 1) * 8192],
                            XI[:, bti * BT * 1024 + q * 8192:
                               bti * BT * 1024 + (q + 1) * 8192])

                # ---- grids; pads carry the per-channel -T/S fold value.
                # Pads are identical each iteration, so fill them only on the
                # first pass through each of the 2 rotating slots.
                h1 = actp.tile([128, BT * 1024], f16, name="h1")
                h1v = h1.rearrange("p (i r w) -> p i r w", r=32, w=32)
                h3 = actp.tile([128, BT * 256], f16, name="h3")
                h3v = h3.rearrange("p (i r w) -> p i r w", r=16, w=16)
                h4 = actp.tile([128, BT * 81], f16, name="h4")
                h4v = h4.rearrange("p (i r w) -> p i r w", r=9, w=9)
                h5t = actp.tile([128, 2, BT, 3, 3], f16, name="h5t")
                if bti < 2:
                    pc = aff[:, 39:40]
                    nc.gpsimd.tensor_copy(h1v[:, :, 0:2, :], pc.to_broadcast((128, BT, 2, 32)))
                    nc.gpsimd.tensor_copy(h1v[:, :, 30:32, :], pc.to_broadcast((128, BT, 2, 32)))
                    nc.gpsimd.tensor_copy(h1v[:, :, 2:30, 0:2], pc.to_broadcast((128, BT, 28, 2)))
                    nc.gpsimd.tensor_copy(h1v[:, :, 2:30, 30:32], pc.to_broadcast((128, BT, 28, 2)))
                    pc = aff[:, 40:41]
                    nc.gpsimd.tensor_copy(h3v[:, :, 0:1, :], pc.to_broadcast((128, BT, 1, 16)))
                    nc.gpsimd.tensor_copy(h3v[:, :, 15:16, :], pc.to_broadcast((128, BT, 1, 16)))
                    nc.gpsimd.tensor_copy(h3v[:, :, 1:15, 0:1], pc.to_broadcast((128, BT, 14, 1)))
                    nc.gpsimd.tensor_copy(h3v[:, :, 1:15, 15:16], pc.to_broadcast((128, BT, 14, 1)))
                    pc = aff[:, 41:42]
                    nc.gpsimd.tensor_copy(h4v[:, :, 0:1, :], pc.to_broadcast((128, BT, 1, 9)))
                    nc.gpsimd.tensor_copy(h4v[:, :, 8:9, :], pc.to_broadcast((128, BT, 1, 9)))
                    nc.gpsimd.tensor_copy(h4v[:, :, 1:8, 0:1], pc.to_broadcast((128, BT, 7, 1)))
                    nc.gpsimd.tensor_copy(h4v[:, :, 1:8, 8:9], pc.to_broadcast((128, BT, 7, 1)))

                # ==== conv1 (whole tile), then conv2 ====
                xiv = xi_sb.rearrange("p (i r w) -> p i r w", r=32, w=32)
                for ss in range(BT // 4):
                    for cb in range(2):
                        ps1 = [psp.tile([128, 392], f32, name=f"ps1_{cb}_{g}",
                                        tag="ps") for g in range(4)]
                        for g in range(4):
                            img = ss * 4 + g
                            nc.tensor.matmul(
                                ps1[g][:],
                                w1_sb[32 * g:32 * g + 25, :],
                                xiv[32 * g:32 * g + 25, img, 14 * cb:14 * cb + 14, 0:28],
                                start=True, stop=True, tile_position=(32 * g, 0))
                        for g in range(4):
                            img = ss * 4 + g
                            dst = h1v[:, img, 2 + 14 * cb:16 + 14 * cb, 2:30]
                            psr = ps1[g].rearrange("p (r w) -> p r w", w=28)
                            nc.scalar.copy(dst, psr)
                            nc.vector.tensor_scalar(dst, dst, aff[:, 0:1],
                                                    aff[:, 1:2], MAX, MIN)
                for ss in range(BT // 4):
                    hps = {}
                    for cb in range(2):
                        ps2 = [psp.tile([128, 392], f32, name=f"ps2_{cb}_{g}",
                                        tag="ps") for g in range(4)]
                        for pos in range(25):
                            kh, kw = divmod(pos, 5)
                            for g in range(4):
                                img = ss * 4 + g
                                nc.tensor.matmul(
                                    ps2[g][:],
                                    w2_sb[32 * g:32 * g + 32, pos, :],
                                    h1v[32 * g:32 * g + 32, img,
                                        14 * cb + kh:14 * cb + kh + 14, kw:kw + 28],
                                    start=(pos == 0), stop=(pos == 24),
                                    tile_position=(32 * g, 0))
                        for g in range(4):
                            img = ss * 4 + g
                            if cb == 0:
                                hps[g] = tmpp.tile([128, 14, 14], f16, name="hp2",
                                                   bufs=6)
                            psv = ps2[g].rearrange(
                                "p (y ty x tx) -> p y x ty tx", y=7, ty=2, tx=2)
                            nc.vector.tensor_reduce(
                                hps[g][:, 7 * cb:7 * cb + 7, :], psv[:],
                                axis=XY, op=MAX)
                            if cb == 1:
                                nc.vector.tensor_scalar(
                                    h3v[:, img, 1:15, 1:15], hps[g][:],
                                    aff[:, 2:3], aff[:, 3:4], MAX, MIN)

                # ======== conv3: 9 shifted MMs, 2 strips x 2 banks/strip ========
                for ss in range(2):
                    pss = [psp.tile([128, 392], f32, name=f"ps3_{g}_{cb}", tag="ps")
                           for g in range(2) for cb in range(2)]
                    for pos in range(9):
                        kh, kw = divmod(pos, 3)
                        for g in range(2):
                            for cb in range(2):
                                i0 = ss * 8 + 4 * cb + 2 * g  # first of 2 images
                                nc.tensor.matmul(
                                    pss[2 * g + cb][:],
                                    w3_sb[64 * g:64 * g + 64, pos, :],
                                    h3v[64 * g:64 * g + 64, i0:i0 + 2,
                                        kh:kh + 14, kw:kw + 14],
                                    start=(pos == 0), stop=(pos == 8),
                                    tile_position=(64 * g, 0))
                    for g in range(2):
                        for cb in range(2):
                            i0 = ss * 8 + 4 * cb + 2 * g
                            psv = pss[2 * g + cb].rearrange(
                                "p (m ty x tx) -> p m x ty tx", m=14, ty=2, tx=2)
                            hp3 = tmpp.tile([128, 14, 7], f16, name="hp3")
                            nc.vector.tensor_reduce(hp3[:], psv[:], axis=XY, op=MAX)
                            nc.vector.tensor_scalar(
                                h4v[:, i0:i0 + 2, 1:8, 1:8],
                                hp3.rearrange("p (i y) x -> p i y x", i=2),
                                aff[:, 4:5], aff[:, 5:6], MAX, MIN)

                # ======== conv4: K=128, M=256 in halves, 3 psum chunks ========
                pse = {(mh, img0): psp.tile([128, 490], f32,
                                            name=f"ps4_{mh}_{img0}", tag="ps")
                       for mh in range(2) for img0 in (0, 10)}
                for pos in range(9):
                    kh, kw = divmod(pos, 3)
                    for mh in range(2):
                        for (img0, nimg) in ((0, 10), (10, 6)):
                            nc.tensor.matmul(
                                pse[(mh, img0)][:, 0:nimg * 49],
                                w4_sb[:, pos, 128 * mh:128 * mh + 128],
                                h4v[:, img0:img0 + nimg, kh:kh + 7, kw:kw + 7],
                                start=(pos == 0), stop=(pos == 8))
                for mh in range(2):
                    for (img0, nimg) in ((0, 10), (10, 6)):
                        psv = pse[(mh, img0)][:, 0:nimg * 49].rearrange(
                            "p (i r w) -> p i r w", r=7, w=7)
                        tw4 = tmpp.tile([128, 10, 6, 3], f16, name="tw4")
                        nc.vector.tensor_reduce(
                            tw4[:, 0:nimg],
                            psv[:, :, 0:6, 0:6].rearrange("p i r (x t) -> p i r x t", t=2),
                            axis=AX, op=MAX)
                        twp4 = tw4[:, 0:nimg].rearrange("p i (r t) x -> p i r t x", t=2)
                        hp4 = tmpp.tile([128, 10, 3, 3], f16, name="hp4")
                        nc.vector.tensor_tensor(hp4[:, 0:nimg], twp4[:, :, :, 0, :],
                                                twp4[:, :, :, 1, :], MAX)
                        nc.vector.tensor_scalar(
                            h5t[:, mh, img0:img0 + nimg, :, :], hp4[:, 0:nimg],
                            aff[:, 6 + mh:7 + mh], aff[:, 8 + mh:9 + mh], MAX, MIN)

                # ======== conv5 (1x1): K=256 in 2 chunks, M=256 in halves ======
                for mh in range(2):
                    ps5 = psp.tile([128, BT * 9], f32, name="ps5", tag="ps")
                    for kc in range(2):
                        nc.tensor.matmul(
                            ps5[:],
                            w5_sb[:, kc, 128 * mh:128 * mh + 128],
                            h5t[:, kc, :, :, :],
                            start=(kc == 0), stop=(kc == 1))
                    nc.vector.tensor_scalar(
                        h5f[:, mh, :, bti * BT:(bti + 1) * BT],
                        ps5.rearrange("p (i s) -> p s i", s=9),
                        aff[:, 10 + mh:11 + mh], aff[:, 12 + mh:13 + mh], MAX, MIN)

            # ================= FC phase (batch = bc) =================
            z1 = bigp.tile([128, 8, bc], f16)
            fpt = [psp.tile([128, bc], f32, name=f"fpt{i}", tag="ps")
                   for i in range(8)]
            fps = [fpt[mt][:] for mt in range(8)]
            for kt in range(18):
                s_, cc = divmod(kt, 2)
                fw1t = fwp.tile([128, 1024], f16, name="fw1t")
                nc.scalar.dma_start(fw1t[:], FW1[kt])
                for mt in range(8):
                    nc.tensor.matmul(fps[mt], fw1t[:, 128 * mt:128 * mt + 128],
                                     h5f[:, cc, s_, :],
                                     start=(kt == 0), stop=(kt == 17))
            for mt in range(8):
                nc.vector.tensor_scalar(z1[:, mt, :], fps[mt],
                                        aff[:, 14 + mt:15 + mt],
                                        aff[:, 22 + mt:23 + mt], MAX, MIN)

            z2 = bigp.tile([128, 4, bc], f16)
            f2t = [psp.tile([128, bc], f32, name=f"f2t{i}", tag="ps") for i in range(4)]
            f2s = [f2t[mt][:] for mt in range(4)]
            for kt in range(8):
                fw2t = fwp.tile([128, 512], f16, name="fw2t")
                nc.scalar.dma_start(fw2t[:], FW2[kt])
                for mt in range(4):
                    nc.tensor.matmul(f2s[mt], fw2t[:, 128 * mt:128 * mt + 128],
                                     z1[:, kt, :],
                                     start=(kt == 0), stop=(kt == 7))
            for mt in range(4):
                nc.vector.tensor_scalar(z2[:, mt, :], f2s[mt],
                                        aff[:, 30 + mt:31 + mt],
                                        aff[:, 34 + mt:35 + mt], MAX, MIN)

            f3ps = psp.tile([10, bc], f32, name="f3ps", tag="ps")
            for kt in range(4):
                nc.tensor.matmul(f3ps[:], fw3_sb[:, kt, :], z2[:, kt, :],
                                 start=(kt == 0), stop=(kt == 3))
            out_sb = tmpp.tile([10, bc], f32, name="out_sb")
            nc.scalar.activation(out_sb[:], f3ps[:], IDENT,
                                 bias=aff[0:10, 38:39], scale=1.0)
            nc.sync.dma_start(OUT[:].rearrange("b c -> c b"), out_sb[:])

    nc.compile()
    return nc


def _get_nc(nbt):
    if nbt not in _NC_CACHE:
        _NC_CACHE[nbt] = _build_module(nbt)
    return _NC_CACHE[nbt]


def _make_in_maps(inputs, ncores=NCORES, bc=BCORE):
    consts = _prep_consts(inputs)
    x = np.asarray(inputs["x"], np.float32)
    in_maps = []
    for c in range(ncores):
        m = dict(consts)
        m["XI"] = _prep_xi(x[c * bc:(c + 1) * bc])
        in_maps.append(m)
    return in_maps


def run(inputs, trace=False, tmpdir=None, ncores=NCORES):
    from concourse.bass_utils import run_bass_kernel_spmd

    nbt = BCORE // BT
    nc = _get_nc(nbt)
    in_maps = _make_in_maps(inputs, ncores=ncores)
    kw = {}
    if trace:
        if tmpdir is not None:
            os.makedirs(tmpdir, exist_ok=True)
        kw = dict(trace=True, tmpdir=tmpdir)
    res = run_bass_kernel_spmd(nc, in_maps, core_ids=list(range(ncores)), **kw)
    out = np.concatenate([res.results[c]["OUT"] for c in range(ncores)], axis=0)
    return out.astype(np.float32), res


def kernel(**inputs):
    out, _ = run(inputs)
    return out
